# revision 23
# baseline (speedup 1.0000x reference)
"""Multi-head attention layer on 8 Trainium2 NeuronCores.

Problem: B=4, S=2048, D=1024, H=16 heads (hd=64), fp32 in/out.

Sharding: core c -> (batch b = c//2, head-group g = c%2). Each core computes
8 heads of one batch element. Fully data/tensor-parallel; no collectives.

Per-core dataflow (bf16 matmuls, fp32 PSUM accumulation):
  - host ships x[b].T (D on partitions) and W[g-slice].T, cast to bf16
  - qT = (x @ Wq_g.T).T and kT likewise: [512, S] with head-dim on partitions
    -> head h occupies 64 partitions; head pairs share a 128-partition chunk
  - v  = x @ Wv_g.T natural [S, 512], stored per-head with a ones column
    appended ([S, 8, 65]) for the softmax-rowsum trick
  - scores computed TRANSPOSED: P.T[k, q] = sum_d kT[d,k] qT[d,q], so softmax
    renormalization needs a k-sum = partition-dim sum, obtained for free as a
    65th matmul output row (ones column of v). Head pairs run as row-tiled
    concurrent matmuls (K=64 each at partition bases 0/64).
  - exp on ScalarE straight out of PSUM (3-bank supertiles to amortize the
    per-instruction overhead); no max subtraction (logits ~ N(0,1))
  - h.T[d, q] (+ rowsum row) accumulated over k-chunks in PSUM, DMA'd out
    unnormalized; host divides by the rowsum and reassembles.
"""

import sys

sys.path.insert(0, "/opt/trn_rl_repo")

from contextlib import ExitStack

import ml_dtypes
import numpy as np

import concourse.bass as bass
import concourse.tile as tile
from concourse import bacc, mybir
from concourse.bass_utils import run_bass_kernel_spmd

F32 = mybir.dt.float32
I16 = mybir.dt.int16
BF16 = mybir.dt.bfloat16
EXP = mybir.ActivationFunctionType.Exp

B, S, D, H = 4, 2048, 1024, 16
HD = D // H          # 64
DG = D // 2          # 512 features per head-group (8 heads)
P = 128
KC = D // P          # 8 contraction chunks
NB = S // 512        # 4 token blocks of 512
NT = S // P          # 16 token tiles of 128
LH = 8               # local heads per core
G = 2
# Schraudolph exp2 constants in int16/bf16: i16 = s*(2^7/(8 ln2)) +
# (127*2^7 - C); bitcasting the int16 as bf16 gives ~= exp(s/8) in one DVE
# op (|rel err| < 4.2%, ~zero mean; C tuned empirically).
EXP2_A = float((1 << 7) / (8.0 * np.log(2.0)))
EXP2_B = float(127.0 * (1 << 7) - 7.25)                # exp supertile: score slices per ACT instruction


def _build_attention(tc: tile.TileContext, ctx: ExitStack, io):
    nc = tc.nc
    xT, wqT, wkT, wvT, bq, bk, bv, out = io

    const_pool = ctx.enter_context(tc.tile_pool(name="const", bufs=1))
    big_pool = ctx.enter_context(tc.tile_pool(name="big", bufs=1))
    pt_pool = ctx.enter_context(tc.tile_pool(name="ptp", bufs=32))
    ht_pool = ctx.enter_context(tc.tile_pool(name="htp", bufs=4))
    # PSUM: 6 banks for score supertiles + 2 banks shared between the
    # projection accumulators and the pv accumulators (same pool+tag), so
    # attention can start while projections are still in flight.
    psum_sc = ctx.enter_context(
        tc.tile_pool(name="psc", bufs=3, space=bass.MemorySpace.PSUM)
    )
    psum_pv = ctx.enter_context(
        tc.tile_pool(name="ppv", bufs=2, space=bass.MemorySpace.PSUM)
    )

    # biases: bq/bk arrive partition-major [128, 4] (feature f = mt*128 + p)
    # for per-partition tensor_scalar adds; bv arrives broadcast [128, 512]
    bq_sb = const_pool.tile([P, 4], F32, name="bq_sb", tag="bq")
    bk_sb = const_pool.tile([P, 4], F32, name="bk_sb", tag="bk")
    bv_sb = const_pool.tile([P, LH, HD], BF16, name="bv_sb", tag="bv")
    nc.sync.dma_start(bq_sb[:], bq[:])
    nc.sync.dma_start(bk_sb[:], bk[:])
    nc.sync.dma_start(bv_sb[:], bv.rearrange("p (l h) -> p l h", l=LH))

    x_sb = big_pool.tile([P, KC, S], BF16, name="x_sb", tag="x_sb")
    wq_sb = big_pool.tile([P, KC, DG], BF16, name="wq_sb", tag="wq_sb")
    wk_sb = big_pool.tile([P, KC, DG], BF16, name="wk_sb", tag="wk_sb")
    wv_sb = big_pool.tile([P, KC, DG], BF16, name="wv_sb", tag="wv_sb")
    # DMA in dependency order with one large transfer each: q/k weights,
    # then x one token-block at a time (the first q/k projection tiles only
    # need block nb0), then v weights.
    nc.sync.dma_start(wq_sb[:], wqT.rearrange("(c p) d -> p c d", p=P))
    nc.sync.dma_start(
        x_sb[:, :, 0:512], xT[:, 0:512].rearrange("(c p) n -> p c n", p=P)
    )
    nc.sync.dma_start(wk_sb[:], wkT.rearrange("(c p) d -> p c d", p=P))
    for nb in range(1, NB):
        nc.sync.dma_start(
            x_sb[:, :, nb * 512 : (nb + 1) * 512],
            xT[:, nb * 512 : (nb + 1) * 512].rearrange("(c p) n -> p c n", p=P),
        )
    nc.sync.dma_start(wv_sb[:], wvT.rearrange("(c p) d -> p c d", p=P))

    qT_sb = big_pool.tile([P, 4, S], BF16, name="qT_sb", tag="qT_sb")
    kT_sb = big_pool.tile([P, 4, S], BF16, name="kT_sb", tag="kT_sb")
    v_sb = big_pool.tile([P, NT, LH, HD + 1], BF16, name="v_sb", tag="v_sb")
    nc.vector.memset(v_sb[:, :, :, HD : HD + 1], 1.0)

    def proj_qk(w_sb, b_sb, dst_sb, mt, nb, pool, tag):
        # dst.T tile: rows = W-slice features (mt), cols = tokens
        ps = pool.tile([P, 512], F32, name="ps_qk", tag=tag)
        for c in range(KC):
            nc.tensor.matmul(
                ps[:],
                lhsT=w_sb[:, c, mt * P : (mt + 1) * P],
                rhs=x_sb[:, c, nb * 512 : (nb + 1) * 512],
                start=(c == 0),
                stop=(c == KC - 1),
            )
        nc.vector.tensor_scalar_add(
            dst_sb[:, mt, nb * 512 : (nb + 1) * 512], ps[:], b_sb[:, mt : mt + 1]
        )

    def proj_v(tt, pool, tag):
        ps_v = pool.tile([P, LH, HD], F32, name="ps_v", tag=tag)
        for c in range(KC):
            nc.tensor.matmul(
                ps_v[:],
                lhsT=x_sb[:, c, tt * P : (tt + 1) * P],
                rhs=wv_sb[:, c, :],
                start=(c == 0),
                stop=(c == KC - 1),
            )
        nc.vector.tensor_add(v_sb[:, tt, :, 0:HD], ps_v[:], bv_sb[:])

    # Upfront on the ppv banks (free until the first pv accumulators exist):
    # just enough q/k projection to unblock the first score matmuls.
    proj_qk(wq_sb, bq_sb, qT_sb, 0, 0, psum_pv, "pv")
    proj_qk(wk_sb, bk_sb, kT_sb, 0, 0, psum_pv, "pv")

    # Everything else is background work emitted into the attention stream by
    # deadline (in units of global score-supertiles). Background tiles borrow
    # score-supertile PSUM slots (bufs=3 keeps exp double-buffered) — never
    # the ppv slots, which pv accumulators hold for a whole iteration.
    NST = NT  # supertiles per (hp, qb) iteration; st j == key tile kt=j
    BATCH = 4  # supertiles of scores emitted back-to-back before their pv group
    bg = []
    for nb in range(1, NB):
        bg.append((4 * nb, lambda nb=nb: proj_qk(wk_sb, bk_sb, kT_sb, 0, nb, psum_sc, "sc")))
        bg.append((NST * nb, lambda nb=nb: proj_qk(wq_sb, bq_sb, qT_sb, 0, nb, psum_sc, "sc")))
    for tt in range(NT):
        # v[kt] is first read by the pv of iteration (0,0), which runs
        # cascaded during iteration (0,1): global st NST + kt
        bg.append((NST + tt - 4, lambda tt=tt: proj_v(tt, psum_sc, "sc")))
    for mt in range(1, 4):
        for nb in range(NB):
            bg.append((4 * NST * mt + 4 * nb - 13, lambda mt=mt, nb=nb: proj_qk(wk_sb, bk_sb, kT_sb, mt, nb, psum_sc, "sc")))
            bg.append((4 * NST * mt + NST * nb - 9, lambda mt=mt, nb=nb: proj_qk(wq_sb, bq_sb, qT_sb, mt, nb, psum_sc, "sc")))
    bg.sort(key=lambda t: t[0])
    bg_pos = 0
    stg = 0

    def drain_bg():
        nonlocal bg_pos
        while bg_pos < len(bg) and bg[bg_pos][0] <= stg + 6:
            bg[bg_pos][1]()
            bg_pos += 1

    def emit_pv_batch(pv_ps, hp, kt, pt_st):
        for h2 in range(2):
            nc.tensor.matmul(
                pv_ps[h2][:],
                lhsT=v_sb[:, kt, hp * 2 + h2, :],
                rhs=pt_st[:, h2, :],
                start=(kt == 0),
                stop=(kt == NT - 1),
            )

    def emit_evac(pv_ps, hp, qb):
        for h2 in range(2):
            lh = hp * 2 + h2
            ht_stage = ht_pool.tile([HD + 1, 512], F32, name="ht_stage", tag="ht")
            nc.vector.tensor_copy(ht_stage[:], pv_ps[h2][:])
            nc.sync.dma_start(out[lh, :, qb * 512 : (qb + 1) * 512], ht_stage[:])

    # Attention, pv cascaded by one iteration: while iteration it streams its
    # scores+exp, the pv accumulation of iteration it-1 interleaves (one kt
    # batch per supertile), so its P.T tiles are already in SBUF and the early
    # iterations have PE slack for the background projections.
    prev = None  # (hp, qb, [pt_st per kt])
    for it in range(16):
        hp, qb = divmod(it, 4)
        if prev is not None:
            prev_ps = [
                psum_pv.tile([HD + 1, 512], F32, name=f"pvh{h2}", tag="pv")
                for h2 in range(2)
            ]
        cur_pts = []
        for kt0 in range(0, NST, BATCH):
            # two supertiles of scores back-to-back, then the two matching pv
            # batches of the previous iteration: full-row matmul chains keep
            # the PE background weight buffer streaming, and grouping halves
            # the row-tiled<->full-row transitions that break it
            for kt in range(kt0, kt0 + BATCH):
                ps_st = psum_sc.tile([P, 2, 512], F32, name="ps_st", tag="sc")
                pt_st = pt_pool.tile([P, 2, 512], BF16, name="pt_st", tag="pt")
                for h2 in range(2):
                    base = h2 * 64
                    nc.tensor.matmul(
                        ps_st[:, h2, :],
                        lhsT=kT_sb[base : base + 64, hp, kt * P : (kt + 1) * P],
                        rhs=qT_sb[base : base + 64, hp, qb * 512 : (qb + 1) * 512],
                        start=True,
                        stop=True,
                    )
                if stg % 8 == 5:
                    # offload an eighth of the exp work to the otherwise-idle
                    # VectorE: Schraudolph exp2 computed directly into the
                    # bf16 bit pattern (int16 view), one DVE op per supertile
                    nc.vector.tensor_scalar(
                        pt_st[:].bitcast(I16),
                        ps_st[:],
                        EXP2_A,
                        EXP2_B,
                        mybir.AluOpType.mult,
                        mybir.AluOpType.add,
                    )
                else:
                    nc.scalar.activation(pt_st[:], ps_st[:], EXP, scale=1.0 / 8.0)
                cur_pts.append(pt_st)
                stg += 1
            if prev is not None:
                for kt in range(kt0, kt0 + BATCH):
                    emit_pv_batch(prev_ps, prev[0], kt, prev[2][kt])
            drain_bg()
        if prev is not None:
            emit_evac(prev_ps, prev[0], prev[1])
        prev = (hp, qb, cur_pts)

    # tail: the last iteration's pv has no successor to cascade into
    last_ps = [
        psum_pv.tile([HD + 1, 512], F32, name=f"pvl{h2}", tag="pv")
        for h2 in range(2)
    ]
    for kt in range(NST):
        emit_pv_batch(last_ps, prev[0], kt, prev[2][kt])
    emit_evac(last_ps, prev[0], prev[1])


def build_program():
    nc = bacc.Bacc(
        "TRN2", target_bir_lowering=False, debug=False, num_devices=8
    )
    xT = nc.dram_tensor("xT", [D, S], BF16, kind="ExternalInput").ap()
    wqT = nc.dram_tensor("wqT", [D, DG], BF16, kind="ExternalInput").ap()
    wkT = nc.dram_tensor("wkT", [D, DG], BF16, kind="ExternalInput").ap()
    wvT = nc.dram_tensor("wvT", [D, DG], BF16, kind="ExternalInput").ap()
    bq = nc.dram_tensor("bq", [P, 4], F32, kind="ExternalInput").ap()
    bk = nc.dram_tensor("bk", [P, 4], F32, kind="ExternalInput").ap()
    bv = nc.dram_tensor("bv", [P, DG], BF16, kind="ExternalInput").ap()
    out = nc.dram_tensor("out", [LH, HD + 1, S], F32, kind="ExternalOutput").ap()

    with tile.TileContext(nc) as tc, ExitStack() as ctx:
        _build_attention(tc, ctx, (xT, wqT, wkT, wvT, bq, bk, bv, out))
    nc.compile()
    return nc


def make_in_maps(x, Wq, bq, Wk, bk, Wv, bv):
    bf = ml_dtypes.bfloat16
    x = np.asarray(x, np.float32)
    in_maps = []
    for c in range(8):
        b, g = c // 2, c % 2
        sl = slice(DG * g, DG * (g + 1))
        in_maps.append(
            {
                "xT": np.ascontiguousarray(x[b].T).astype(bf),
                "wqT": np.ascontiguousarray(np.asarray(Wq, np.float32)[sl].T).astype(bf),
                "wkT": np.ascontiguousarray(np.asarray(Wk, np.float32)[sl].T).astype(bf),
                "wvT": np.ascontiguousarray(np.asarray(Wv, np.float32)[sl].T).astype(bf),
                "bq": np.ascontiguousarray(np.asarray(bq, np.float32)[sl].reshape(4, P).T),
                "bk": np.ascontiguousarray(np.asarray(bk, np.float32)[sl].reshape(4, P).T),
                "bv": np.ascontiguousarray(
                    np.broadcast_to(np.asarray(bv, np.float32)[sl], (P, DG))
                ).astype(bf),
            }
        )
    return in_maps


def assemble(outs):
    res = np.empty((B, S, D), np.float32)
    for c in range(8):
        b, g = c // 2, c % 2
        o = np.asarray(outs[c], np.float32)       # [8, 65, 2048]
        hn = o[:, :HD, :] / o[:, HD : HD + 1, :]  # normalize softmax
        res[b, :, DG * g : DG * (g + 1)] = hn.transpose(2, 0, 1).reshape(S, DG)
    return res


_NC_CACHE = None


def _get_program():
    global _NC_CACHE
    if _NC_CACHE is None:
        _NC_CACHE = build_program()
    return _NC_CACHE


def _install_ntff_hook():
    """The agent image lacks ``antenv.axon_hooks``; recreate it and install
    the ctypes NTFF-profiling hook against libaxon_pjrt.so (the same thing
    trn_boot does when the module exists). Only used for trace=True runs."""
    import contextlib
    import ctypes
    import types

    try:
        from antenv.axon_hooks import get_axon_ntff_profile_hook  # noqa: F401

        return
    except ImportError:
        pass

    so_path = "/opt/axon/libaxon_pjrt.so"
    lib = ctypes.CDLL(so_path)
    if not hasattr(lib, "axon_start_nrt_profile"):
        return
    lib.axon_start_nrt_profile.argtypes = [
        ctypes.POINTER(ctypes.c_int64),
        ctypes.c_size_t,
    ]
    lib.axon_start_nrt_profile.restype = ctypes.c_int64
    lib.axon_stop_nrt_profile.argtypes = [ctypes.c_char_p]
    lib.axon_stop_nrt_profile.restype = ctypes.c_int64

    @contextlib.contextmanager
    def _hook(output_dir, device_ids):
        import jax

        jax.devices()
        if device_ids:
            ids = (ctypes.c_int64 * len(device_ids))(*device_ids)
            rc = lib.axon_start_nrt_profile(ids, len(device_ids))
        else:
            rc = lib.axon_start_nrt_profile(None, 0)
        if rc != 0:
            raise RuntimeError(f"axon_start_nrt_profile rc={rc}")
        try:
            yield
        finally:
            n = lib.axon_stop_nrt_profile(str(output_dir).encode())
            print(f"ntff profile: {n} file(s) written to {output_dir}")

    mod = types.ModuleType("antenv.axon_hooks")
    mod._hook = _hook
    mod.set_axon_ntff_profile_hook = lambda h: setattr(mod, "_hook", h)
    mod.get_axon_ntff_profile_hook = lambda: mod._hook
    sys.modules["antenv.axon_hooks"] = mod

    # artifact upload reaches for a shared bucket that this container can't
    # see; the local tmpdir is all the profile pipeline needs
    import concourse.bass_utils as bu

    bu.upload_artifacts = lambda tmpdir: tmpdir


def kernel(x, Wq, bq, Wk, bk, Wv, bv, trace=False, tmpdir=None):
    nc = _get_program()
    if trace:
        _install_ntff_hook()
    in_maps = make_in_maps(x, Wq, bq, Wk, bk, Wv, bv)
    res = run_bass_kernel_spmd(
        nc, in_maps, core_ids=list(range(8)), trace=trace, tmpdir=tmpdir
    )
    full = assemble([res.results[c]["out"] for c in range(8)])
    if trace:
        kernel.last_results = res
    return full


# revision 25
# speedup vs baseline: 1.0187x; 1.0187x over previous
"""Multi-head attention layer on 8 Trainium2 NeuronCores.

Problem: B=4, S=2048, D=1024, H=16 heads (hd=64), fp32 in/out.

Sharding: core c -> (batch b = c//2, head-group g = c%2). Each core computes
8 heads of one batch element. Fully data/tensor-parallel; no collectives.

Per-core dataflow (bf16 matmuls, fp32 PSUM accumulation):
  - host ships x[b].T (D on partitions) and W[g-slice].T, cast to bf16
  - qT = (x @ Wq_g.T).T and kT likewise: [512, S] with head-dim on partitions
    -> head h occupies 64 partitions; head pairs share a 128-partition chunk
  - v  = x @ Wv_g.T natural [S, 512], stored per-head with a ones column
    appended ([S, 8, 65]) for the softmax-rowsum trick
  - scores computed TRANSPOSED: P.T[k, q] = sum_d kT[d,k] qT[d,q], so softmax
    renormalization needs a k-sum = partition-dim sum, obtained for free as a
    65th matmul output row (ones column of v). Head pairs run as row-tiled
    concurrent matmuls (K=64 each at partition bases 0/64).
  - exp on ScalarE straight out of PSUM (3-bank supertiles to amortize the
    per-instruction overhead); no max subtraction (logits ~ N(0,1))
  - h.T[d, q] (+ rowsum row) accumulated over k-chunks in PSUM, DMA'd out
    unnormalized; host divides by the rowsum and reassembles.
"""

import sys

sys.path.insert(0, "/opt/trn_rl_repo")

from contextlib import ExitStack

import ml_dtypes
import numpy as np

import concourse.bass as bass
import concourse.tile as tile
from concourse import bacc, mybir
from concourse.bass_utils import run_bass_kernel_spmd

F32 = mybir.dt.float32
I16 = mybir.dt.int16
BF16 = mybir.dt.bfloat16
EXP = mybir.ActivationFunctionType.Exp

B, S, D, H = 4, 2048, 1024, 16
HD = D // H          # 64
DG = D // 2          # 512 features per head-group (8 heads)
P = 128
KC = D // P          # 8 contraction chunks
NB = S // 512        # 4 token blocks of 512
NT = S // P          # 16 token tiles of 128
LH = 8               # local heads per core
G = 2
# Schraudolph exp2 constants in int16/bf16: i16 = s*(2^7/(8 ln2)) +
# (127*2^7 - C); bitcasting the int16 as bf16 gives ~= exp(s/8) in one DVE
# op (|rel err| < 4.2%, ~zero mean; C tuned empirically).
EXP2_A = float((1 << 7) / (8.0 * np.log(2.0)))
EXP2_B = float(127.0 * (1 << 7) - 7.25)                # exp supertile: score slices per ACT instruction


def _build_attention(tc: tile.TileContext, ctx: ExitStack, io):
    nc = tc.nc
    xT, wqT, wkT, wvT, bq, bk, bv, out = io

    const_pool = ctx.enter_context(tc.tile_pool(name="const", bufs=1))
    big_pool = ctx.enter_context(tc.tile_pool(name="big", bufs=1))
    pt_pool = ctx.enter_context(tc.tile_pool(name="ptp", bufs=32))
    ht_pool = ctx.enter_context(tc.tile_pool(name="htp", bufs=4))
    # PSUM: 6 banks for score supertiles + 2 banks shared between the
    # projection accumulators and the pv accumulators (same pool+tag), so
    # attention can start while projections are still in flight.
    psum_sc = ctx.enter_context(
        tc.tile_pool(name="psc", bufs=3, space=bass.MemorySpace.PSUM)
    )
    psum_pv = ctx.enter_context(
        tc.tile_pool(name="ppv", bufs=2, space=bass.MemorySpace.PSUM)
    )

    # biases: bq/bk arrive partition-major [128, 4] (feature f = mt*128 + p)
    # for per-partition tensor_scalar adds; bv arrives broadcast [128, 512]
    bq_sb = const_pool.tile([P, 4], F32, name="bq_sb", tag="bq")
    bk_sb = const_pool.tile([P, 4], F32, name="bk_sb", tag="bk")
    bv_sb = const_pool.tile([P, LH, HD], BF16, name="bv_sb", tag="bv")
    nc.sync.dma_start(bq_sb[:], bq[:])
    nc.sync.dma_start(bk_sb[:], bk[:])
    nc.sync.dma_start(bv_sb[:], bv.rearrange("p (l h) -> p l h", l=LH))

    x_sb = big_pool.tile([P, KC, S], BF16, name="x_sb", tag="x_sb")
    wq_sb = big_pool.tile([P, KC, DG], BF16, name="wq_sb", tag="wq_sb")
    wk_sb = big_pool.tile([P, KC, DG], BF16, name="wk_sb", tag="wk_sb")
    wv_sb = big_pool.tile([P, KC, DG], BF16, name="wv_sb", tag="wv_sb")
    # DMA in dependency order with one large transfer each: q/k weights,
    # then x one token-block at a time (the first q/k projection tiles only
    # need block nb0), then v weights.
    nc.sync.dma_start(wq_sb[:], wqT.rearrange("(c p) d -> p c d", p=P))
    nc.sync.dma_start(
        x_sb[:, :, 0:512], xT[:, 0:512].rearrange("(c p) n -> p c n", p=P)
    )
    nc.sync.dma_start(wk_sb[:], wkT.rearrange("(c p) d -> p c d", p=P))
    for nb in range(1, NB):
        nc.sync.dma_start(
            x_sb[:, :, nb * 512 : (nb + 1) * 512],
            xT[:, nb * 512 : (nb + 1) * 512].rearrange("(c p) n -> p c n", p=P),
        )
    nc.sync.dma_start(wv_sb[:], wvT.rearrange("(c p) d -> p c d", p=P))

    qT_sb = big_pool.tile([P, 4, S], BF16, name="qT_sb", tag="qT_sb")
    kT_sb = big_pool.tile([P, 4, S], BF16, name="kT_sb", tag="kT_sb")
    v_sb = big_pool.tile([P, NT, LH, HD + 1], BF16, name="v_sb", tag="v_sb")
    nc.vector.memset(v_sb[:, :, :, HD : HD + 1], 1.0)

    def proj_qk(w_sb, b_sb, dst_sb, mt, nb, pool, tag):
        # dst.T tile: rows = W-slice features (mt), cols = tokens
        ps = pool.tile([P, 512], F32, name="ps_qk", tag=tag)
        for c in range(KC):
            nc.tensor.matmul(
                ps[:],
                lhsT=w_sb[:, c, mt * P : (mt + 1) * P],
                rhs=x_sb[:, c, nb * 512 : (nb + 1) * 512],
                start=(c == 0),
                stop=(c == KC - 1),
            )
        nc.vector.tensor_scalar_add(
            dst_sb[:, mt, nb * 512 : (nb + 1) * 512], ps[:], b_sb[:, mt : mt + 1]
        )

    def proj_v(tt, pool, tag):
        ps_v = pool.tile([P, LH, HD], F32, name="ps_v", tag=tag)
        for c in range(KC):
            nc.tensor.matmul(
                ps_v[:],
                lhsT=x_sb[:, c, tt * P : (tt + 1) * P],
                rhs=wv_sb[:, c, :],
                start=(c == 0),
                stop=(c == KC - 1),
            )
        nc.vector.tensor_add(v_sb[:, tt, :, 0:HD], ps_v[:], bv_sb[:])

    # Upfront on the ppv banks (free until the first pv accumulators exist):
    # just enough q/k projection to unblock the first score matmuls.
    proj_qk(wq_sb, bq_sb, qT_sb, 0, 0, psum_pv, "pv")
    proj_qk(wk_sb, bk_sb, kT_sb, 0, 0, psum_pv, "pv")

    # Everything else is background work emitted into the attention stream by
    # deadline (in units of global score-supertiles). Background tiles borrow
    # score-supertile PSUM slots (bufs=3 keeps exp double-buffered) — never
    # the ppv slots, which pv accumulators hold for a whole iteration.
    NST = NT  # supertiles per (hp, qb) iteration; st j == key tile kt=j
    BATCH = 3  # supertiles of scores emitted back-to-back before their pv group
    bg = []
    for nb in range(1, NB):
        bg.append((4 * nb, lambda nb=nb: proj_qk(wk_sb, bk_sb, kT_sb, 0, nb, psum_sc, "sc")))
        bg.append((NST * nb, lambda nb=nb: proj_qk(wq_sb, bq_sb, qT_sb, 0, nb, psum_sc, "sc")))
    for tt in range(NT):
        # v[kt] is first read by the pv of iteration (0,0), which runs
        # cascaded during iteration (0,1): global st NST + kt
        bg.append((NST + tt - 4, lambda tt=tt: proj_v(tt, psum_sc, "sc")))
    for mt in range(1, 4):
        for nb in range(NB):
            bg.append((4 * NST * mt + 4 * nb - 13, lambda mt=mt, nb=nb: proj_qk(wk_sb, bk_sb, kT_sb, mt, nb, psum_sc, "sc")))
            bg.append((4 * NST * mt + NST * nb - 9, lambda mt=mt, nb=nb: proj_qk(wq_sb, bq_sb, qT_sb, mt, nb, psum_sc, "sc")))
    bg.sort(key=lambda t: t[0])
    bg_pos = 0
    stg = 0

    def drain_bg():
        nonlocal bg_pos
        while bg_pos < len(bg) and bg[bg_pos][0] <= stg + 6:
            bg[bg_pos][1]()
            bg_pos += 1

    def emit_pv_batch(pv_ps, hp, kt, pt_st):
        for h2 in range(2):
            nc.tensor.matmul(
                pv_ps[h2][:],
                lhsT=v_sb[:, kt, hp * 2 + h2, :],
                rhs=pt_st[:, h2, :],
                start=(kt == 0),
                stop=(kt == NT - 1),
            )

    def emit_evac(pv_ps, hp, qb):
        for h2 in range(2):
            lh = hp * 2 + h2
            ht_stage = ht_pool.tile([HD + 1, 512], F32, name="ht_stage", tag="ht")
            nc.vector.tensor_copy(ht_stage[:], pv_ps[h2][:])
            nc.sync.dma_start(out[lh, :, qb * 512 : (qb + 1) * 512], ht_stage[:])

    # Attention, pv cascaded by one iteration: while iteration it streams its
    # scores+exp, the pv accumulation of iteration it-1 interleaves (one kt
    # batch per supertile), so its P.T tiles are already in SBUF and the early
    # iterations have PE slack for the background projections.
    prev = None  # (hp, qb, [pt_st per kt])
    for it in range(16):
        hp, qb = divmod(it, 4)
        if prev is not None:
            prev_ps = [
                psum_pv.tile([HD + 1, 512], F32, name=f"pvh{h2}", tag="pv")
                for h2 in range(2)
            ]
        cur_pts = []
        for kt0 in range(0, NST, BATCH):
            # two supertiles of scores back-to-back, then the two matching pv
            # batches of the previous iteration: full-row matmul chains keep
            # the PE background weight buffer streaming, and grouping halves
            # the row-tiled<->full-row transitions that break it
            for kt in range(kt0, min(kt0 + BATCH, NST)):
                ps_st = psum_sc.tile([P, 2, 512], F32, name="ps_st", tag="sc")
                pt_st = pt_pool.tile([P, 2, 512], BF16, name="pt_st", tag="pt")
                for h2 in range(2):
                    base = h2 * 64
                    nc.tensor.matmul(
                        ps_st[:, h2, :],
                        lhsT=kT_sb[base : base + 64, hp, kt * P : (kt + 1) * P],
                        rhs=qT_sb[base : base + 64, hp, qb * 512 : (qb + 1) * 512],
                        start=True,
                        stop=True,
                    )
                if stg % 8 == 5:
                    # offload an eighth of the exp work to the otherwise-idle
                    # VectorE: Schraudolph exp2 computed directly into the
                    # bf16 bit pattern (int16 view), one DVE op per supertile
                    nc.vector.tensor_scalar(
                        pt_st[:].bitcast(I16),
                        ps_st[:],
                        EXP2_A,
                        EXP2_B,
                        mybir.AluOpType.mult,
                        mybir.AluOpType.add,
                    )
                else:
                    nc.scalar.activation(pt_st[:], ps_st[:], EXP, scale=1.0 / 8.0)
                cur_pts.append(pt_st)
                stg += 1
            if prev is not None:
                for kt in range(kt0, min(kt0 + BATCH, NST)):
                    emit_pv_batch(prev_ps, prev[0], kt, prev[2][kt])
            drain_bg()
        if prev is not None:
            emit_evac(prev_ps, prev[0], prev[1])
        prev = (hp, qb, cur_pts)

    # tail: the last iteration's pv has no successor to cascade into
    last_ps = [
        psum_pv.tile([HD + 1, 512], F32, name=f"pvl{h2}", tag="pv")
        for h2 in range(2)
    ]
    for kt in range(NST):
        emit_pv_batch(last_ps, prev[0], kt, prev[2][kt])
    emit_evac(last_ps, prev[0], prev[1])


def build_program():
    nc = bacc.Bacc(
        "TRN2", target_bir_lowering=False, debug=False, num_devices=8
    )
    xT = nc.dram_tensor("xT", [D, S], BF16, kind="ExternalInput").ap()
    wqT = nc.dram_tensor("wqT", [D, DG], BF16, kind="ExternalInput").ap()
    wkT = nc.dram_tensor("wkT", [D, DG], BF16, kind="ExternalInput").ap()
    wvT = nc.dram_tensor("wvT", [D, DG], BF16, kind="ExternalInput").ap()
    bq = nc.dram_tensor("bq", [P, 4], F32, kind="ExternalInput").ap()
    bk = nc.dram_tensor("bk", [P, 4], F32, kind="ExternalInput").ap()
    bv = nc.dram_tensor("bv", [P, DG], BF16, kind="ExternalInput").ap()
    out = nc.dram_tensor("out", [LH, HD + 1, S], F32, kind="ExternalOutput").ap()

    with tile.TileContext(nc) as tc, ExitStack() as ctx:
        _build_attention(tc, ctx, (xT, wqT, wkT, wvT, bq, bk, bv, out))
    nc.compile()
    return nc


def make_in_maps(x, Wq, bq, Wk, bk, Wv, bv):
    bf = ml_dtypes.bfloat16
    x = np.asarray(x, np.float32)
    in_maps = []
    for c in range(8):
        b, g = c // 2, c % 2
        sl = slice(DG * g, DG * (g + 1))
        in_maps.append(
            {
                "xT": np.ascontiguousarray(x[b].T).astype(bf),
                "wqT": np.ascontiguousarray(np.asarray(Wq, np.float32)[sl].T).astype(bf),
                "wkT": np.ascontiguousarray(np.asarray(Wk, np.float32)[sl].T).astype(bf),
                "wvT": np.ascontiguousarray(np.asarray(Wv, np.float32)[sl].T).astype(bf),
                "bq": np.ascontiguousarray(np.asarray(bq, np.float32)[sl].reshape(4, P).T),
                "bk": np.ascontiguousarray(np.asarray(bk, np.float32)[sl].reshape(4, P).T),
                "bv": np.ascontiguousarray(
                    np.broadcast_to(np.asarray(bv, np.float32)[sl], (P, DG))
                ).astype(bf),
            }
        )
    return in_maps


def assemble(outs):
    res = np.empty((B, S, D), np.float32)
    for c in range(8):
        b, g = c // 2, c % 2
        o = np.asarray(outs[c], np.float32)       # [8, 65, 2048]
        hn = o[:, :HD, :] / o[:, HD : HD + 1, :]  # normalize softmax
        res[b, :, DG * g : DG * (g + 1)] = hn.transpose(2, 0, 1).reshape(S, DG)
    return res


_NC_CACHE = None


def _get_program():
    global _NC_CACHE
    if _NC_CACHE is None:
        _NC_CACHE = build_program()
    return _NC_CACHE


def _install_ntff_hook():
    """The agent image lacks ``antenv.axon_hooks``; recreate it and install
    the ctypes NTFF-profiling hook against libaxon_pjrt.so (the same thing
    trn_boot does when the module exists). Only used for trace=True runs."""
    import contextlib
    import ctypes
    import types

    try:
        from antenv.axon_hooks import get_axon_ntff_profile_hook  # noqa: F401

        return
    except ImportError:
        pass

    so_path = "/opt/axon/libaxon_pjrt.so"
    lib = ctypes.CDLL(so_path)
    if not hasattr(lib, "axon_start_nrt_profile"):
        return
    lib.axon_start_nrt_profile.argtypes = [
        ctypes.POINTER(ctypes.c_int64),
        ctypes.c_size_t,
    ]
    lib.axon_start_nrt_profile.restype = ctypes.c_int64
    lib.axon_stop_nrt_profile.argtypes = [ctypes.c_char_p]
    lib.axon_stop_nrt_profile.restype = ctypes.c_int64

    @contextlib.contextmanager
    def _hook(output_dir, device_ids):
        import jax

        jax.devices()
        if device_ids:
            ids = (ctypes.c_int64 * len(device_ids))(*device_ids)
            rc = lib.axon_start_nrt_profile(ids, len(device_ids))
        else:
            rc = lib.axon_start_nrt_profile(None, 0)
        if rc != 0:
            raise RuntimeError(f"axon_start_nrt_profile rc={rc}")
        try:
            yield
        finally:
            n = lib.axon_stop_nrt_profile(str(output_dir).encode())
            print(f"ntff profile: {n} file(s) written to {output_dir}")

    mod = types.ModuleType("antenv.axon_hooks")
    mod._hook = _hook
    mod.set_axon_ntff_profile_hook = lambda h: setattr(mod, "_hook", h)
    mod.get_axon_ntff_profile_hook = lambda: mod._hook
    sys.modules["antenv.axon_hooks"] = mod

    # artifact upload reaches for a shared bucket that this container can't
    # see; the local tmpdir is all the profile pipeline needs
    import concourse.bass_utils as bu

    bu.upload_artifacts = lambda tmpdir: tmpdir


def kernel(x, Wq, bq, Wk, bk, Wv, bv, trace=False, tmpdir=None):
    nc = _get_program()
    if trace:
        _install_ntff_hook()
    in_maps = make_in_maps(x, Wq, bq, Wk, bk, Wv, bv)
    res = run_bass_kernel_spmd(
        nc, in_maps, core_ids=list(range(8)), trace=trace, tmpdir=tmpdir
    )
    full = assemble([res.results[c]["out"] for c in range(8)])
    if trace:
        kernel.last_results = res
    return full


# revision 26
# speedup vs baseline: 1.0334x; 1.0144x over previous
"""Multi-head attention layer on 8 Trainium2 NeuronCores.

Problem: B=4, S=2048, D=1024, H=16 heads (hd=64), fp32 in/out.

Sharding: core c -> (batch b = c//2, head-group g = c%2). Each core computes
8 heads of one batch element. Fully data/tensor-parallel; no collectives.

Per-core dataflow (bf16 matmuls, fp32 PSUM accumulation):
  - host ships x[b].T (D on partitions) and W[g-slice].T, cast to bf16
  - qT = (x @ Wq_g.T).T and kT likewise: [512, S] with head-dim on partitions
    -> head h occupies 64 partitions; head pairs share a 128-partition chunk
  - v  = x @ Wv_g.T natural [S, 512], stored per-head with a ones column
    appended ([S, 8, 65]) for the softmax-rowsum trick
  - scores computed TRANSPOSED: P.T[k, q] = sum_d kT[d,k] qT[d,q], so softmax
    renormalization needs a k-sum = partition-dim sum, obtained for free as a
    65th matmul output row (ones column of v). Head pairs run as row-tiled
    concurrent matmuls (K=64 each at partition bases 0/64).
  - exp on ScalarE straight out of PSUM (3-bank supertiles to amortize the
    per-instruction overhead); no max subtraction (logits ~ N(0,1))
  - h.T[d, q] (+ rowsum row) accumulated over k-chunks in PSUM, DMA'd out
    unnormalized; host divides by the rowsum and reassembles.
"""

import sys

sys.path.insert(0, "/opt/trn_rl_repo")

from contextlib import ExitStack

import ml_dtypes
import numpy as np

import concourse.bass as bass
import concourse.tile as tile
from concourse import bacc, mybir
from concourse.bass_utils import run_bass_kernel_spmd

F32 = mybir.dt.float32
I16 = mybir.dt.int16
BF16 = mybir.dt.bfloat16
EXP = mybir.ActivationFunctionType.Exp

B, S, D, H = 4, 2048, 1024, 16
HD = D // H          # 64
DG = D // 2          # 512 features per head-group (8 heads)
P = 128
KC = D // P          # 8 contraction chunks
NB = S // 512        # 4 token blocks of 512
NT = S // P          # 16 token tiles of 128
LH = 8               # local heads per core
G = 2
# Schraudolph exp2 constants in int16/bf16: i16 = s*(2^7/(8 ln2)) +
# (127*2^7 - C); bitcasting the int16 as bf16 gives ~= exp(s/8) in one DVE
# op (|rel err| < 4.2%, ~zero mean; C tuned empirically).
EXP2_A = float((1 << 7) / (8.0 * np.log(2.0)))
EXP2_B = float(127.0 * (1 << 7) - 7.25)                # exp supertile: score slices per ACT instruction


def _build_attention(tc: tile.TileContext, ctx: ExitStack, io):
    nc = tc.nc
    xT, wqT, wkT, wvT, bq, bk, bv, out = io

    const_pool = ctx.enter_context(tc.tile_pool(name="const", bufs=1))
    big_pool = ctx.enter_context(tc.tile_pool(name="big", bufs=1))
    pt_pool = ctx.enter_context(tc.tile_pool(name="ptp", bufs=32))
    ht_pool = ctx.enter_context(tc.tile_pool(name="htp", bufs=4))
    # PSUM: 6 banks for score supertiles + 2 banks shared between the
    # projection accumulators and the pv accumulators (same pool+tag), so
    # attention can start while projections are still in flight.
    psum_sc = ctx.enter_context(
        tc.tile_pool(name="psc", bufs=3, space=bass.MemorySpace.PSUM)
    )
    psum_pv = ctx.enter_context(
        tc.tile_pool(name="ppv", bufs=2, space=bass.MemorySpace.PSUM)
    )

    # biases: bq/bk arrive partition-major [128, 4] (feature f = mt*128 + p)
    # for per-partition tensor_scalar adds; bv arrives broadcast [128, 512]
    bq_sb = const_pool.tile([P, 4], F32, name="bq_sb", tag="bq")
    bk_sb = const_pool.tile([P, 4], F32, name="bk_sb", tag="bk")
    bv_sb = const_pool.tile([P, LH, HD], BF16, name="bv_sb", tag="bv")
    nc.sync.dma_start(bq_sb[:], bq[:])
    nc.sync.dma_start(bk_sb[:], bk[:])
    nc.sync.dma_start(bv_sb[:], bv.rearrange("p (l h) -> p l h", l=LH))

    x_sb = big_pool.tile([P, KC, S], BF16, name="x_sb", tag="x_sb")
    wq_sb = big_pool.tile([P, KC, DG], BF16, name="wq_sb", tag="wq_sb")
    wk_sb = big_pool.tile([P, KC, DG], BF16, name="wk_sb", tag="wk_sb")
    wv_sb = big_pool.tile([P, KC, DG], BF16, name="wv_sb", tag="wv_sb")
    # DMA in dependency order with one large transfer each: q/k weights,
    # then x one token-block at a time (the first q/k projection tiles only
    # need block nb0), then v weights.
    nc.sync.dma_start(wq_sb[:], wqT.rearrange("(c p) d -> p c d", p=P))
    nc.sync.dma_start(
        x_sb[:, :, 0:512], xT[:, 0:512].rearrange("(c p) n -> p c n", p=P)
    )
    nc.sync.dma_start(wk_sb[:], wkT.rearrange("(c p) d -> p c d", p=P))
    for nb in range(1, NB):
        nc.sync.dma_start(
            x_sb[:, :, nb * 512 : (nb + 1) * 512],
            xT[:, nb * 512 : (nb + 1) * 512].rearrange("(c p) n -> p c n", p=P),
        )
    nc.sync.dma_start(wv_sb[:], wvT.rearrange("(c p) d -> p c d", p=P))

    qT_sb = big_pool.tile([P, 4, S], BF16, name="qT_sb", tag="qT_sb")
    kT_sb = big_pool.tile([P, 4, S], BF16, name="kT_sb", tag="kT_sb")
    v_sb = big_pool.tile([P, NT, LH, HD + 1], BF16, name="v_sb", tag="v_sb")
    nc.vector.memset(v_sb[:, :, :, HD : HD + 1], 1.0)

    def proj_qk(w_sb, b_sb, dst_sb, mt, nb, pool, tag):
        # dst.T tile: rows = W-slice features (mt), cols = tokens
        ps = pool.tile([P, 512], F32, name="ps_qk", tag=tag)
        for c in range(KC):
            nc.tensor.matmul(
                ps[:],
                lhsT=w_sb[:, c, mt * P : (mt + 1) * P],
                rhs=x_sb[:, c, nb * 512 : (nb + 1) * 512],
                start=(c == 0),
                stop=(c == KC - 1),
            )
        nc.vector.tensor_scalar_add(
            dst_sb[:, mt, nb * 512 : (nb + 1) * 512], ps[:], b_sb[:, mt : mt + 1]
        )

    def proj_v(tt, pool, tag):
        ps_v = pool.tile([P, LH, HD], F32, name="ps_v", tag=tag)
        for c in range(KC):
            nc.tensor.matmul(
                ps_v[:],
                lhsT=x_sb[:, c, tt * P : (tt + 1) * P],
                rhs=wv_sb[:, c, :],
                start=(c == 0),
                stop=(c == KC - 1),
            )
        nc.vector.tensor_add(v_sb[:, tt, :, 0:HD], ps_v[:], bv_sb[:])

    # Upfront on the ppv banks (free until the first pv accumulators exist):
    # just enough q/k projection to unblock the first score matmuls.
    proj_qk(wq_sb, bq_sb, qT_sb, 0, 0, psum_pv, "pv")
    proj_qk(wk_sb, bk_sb, kT_sb, 0, 0, psum_pv, "pv")

    # Everything else is background work emitted into the attention stream by
    # deadline (in units of global score-supertiles). Background tiles borrow
    # score-supertile PSUM slots (bufs=3 keeps exp double-buffered) — never
    # the ppv slots, which pv accumulators hold for a whole iteration.
    NST = NT  # supertiles per (hp, qb) iteration; st j == key tile kt=j
    BATCH = 2  # supertiles of scores emitted back-to-back before their pv group
    bg = []
    for nb in range(1, NB):
        bg.append((4 * nb, lambda nb=nb: proj_qk(wk_sb, bk_sb, kT_sb, 0, nb, psum_sc, "sc")))
        bg.append((NST * nb, lambda nb=nb: proj_qk(wq_sb, bq_sb, qT_sb, 0, nb, psum_sc, "sc")))
    for tt in range(NT):
        # v[kt] is first read by the pv of iteration (0,0), which runs
        # cascaded during iteration (0,1): global st NST + kt
        bg.append((NST + tt - 4, lambda tt=tt: proj_v(tt, psum_sc, "sc")))
    for mt in range(1, 4):
        for nb in range(NB):
            bg.append((4 * NST * mt + 4 * nb - 13, lambda mt=mt, nb=nb: proj_qk(wk_sb, bk_sb, kT_sb, mt, nb, psum_sc, "sc")))
            bg.append((4 * NST * mt + NST * nb - 9, lambda mt=mt, nb=nb: proj_qk(wq_sb, bq_sb, qT_sb, mt, nb, psum_sc, "sc")))
    bg.sort(key=lambda t: t[0])
    bg_pos = 0
    stg = 0

    def drain_bg():
        nonlocal bg_pos
        while bg_pos < len(bg) and bg[bg_pos][0] <= stg + 6:
            bg[bg_pos][1]()
            bg_pos += 1

    def emit_pv_batch(pv_ps, hp, kt, pt_st):
        for h2 in range(2):
            nc.tensor.matmul(
                pv_ps[h2][:],
                lhsT=v_sb[:, kt, hp * 2 + h2, :],
                rhs=pt_st[:, h2, :],
                start=(kt == 0),
                stop=(kt == NT - 1),
            )

    def emit_evac(pv_ps, hp, qb):
        for h2 in range(2):
            lh = hp * 2 + h2
            ht_stage = ht_pool.tile([HD + 1, 512], F32, name="ht_stage", tag="ht")
            nc.vector.tensor_copy(ht_stage[:], pv_ps[h2][:])
            nc.sync.dma_start(out[lh, :, qb * 512 : (qb + 1) * 512], ht_stage[:])

    # Attention, pv cascaded by one iteration: while iteration it streams its
    # scores+exp, the pv accumulation of iteration it-1 interleaves (one kt
    # batch per supertile), so its P.T tiles are already in SBUF and the early
    # iterations have PE slack for the background projections.
    prev = None  # (hp, qb, [pt_st per kt])
    for it in range(16):
        hp, qb = divmod(it, 4)
        if prev is not None:
            prev_ps = [
                psum_pv.tile([HD + 1, 512], F32, name=f"pvh{h2}", tag="pv")
                for h2 in range(2)
            ]
        cur_pts = []
        for kt0 in range(0, NST, BATCH):
            # two supertiles of scores back-to-back, then the two matching pv
            # batches of the previous iteration: full-row matmul chains keep
            # the PE background weight buffer streaming, and grouping halves
            # the row-tiled<->full-row transitions that break it
            for kt in range(kt0, min(kt0 + BATCH, NST)):
                ps_st = psum_sc.tile([P, 2, 512], F32, name="ps_st", tag="sc")
                pt_st = pt_pool.tile([P, 2, 512], BF16, name="pt_st", tag="pt")
                for h2 in range(2):
                    base = h2 * 64
                    nc.tensor.matmul(
                        ps_st[:, h2, :],
                        lhsT=kT_sb[base : base + 64, hp, kt * P : (kt + 1) * P],
                        rhs=qT_sb[base : base + 64, hp, qb * 512 : (qb + 1) * 512],
                        start=True,
                        stop=True,
                    )
                if stg % 8 == 5:
                    # offload an eighth of the exp work to the otherwise-idle
                    # VectorE: Schraudolph exp2 computed directly into the
                    # bf16 bit pattern (int16 view), one DVE op per supertile
                    nc.vector.tensor_scalar(
                        pt_st[:].bitcast(I16),
                        ps_st[:],
                        EXP2_A,
                        EXP2_B,
                        mybir.AluOpType.mult,
                        mybir.AluOpType.add,
                    )
                else:
                    nc.scalar.activation(pt_st[:], ps_st[:], EXP, scale=1.0 / 8.0)
                cur_pts.append(pt_st)
                stg += 1
            if prev is not None:
                for kt in range(kt0, min(kt0 + BATCH, NST)):
                    emit_pv_batch(prev_ps, prev[0], kt, prev[2][kt])
            drain_bg()
        if prev is not None:
            emit_evac(prev_ps, prev[0], prev[1])
        prev = (hp, qb, cur_pts)

    # tail: the last iteration's pv has no successor to cascade into
    last_ps = [
        psum_pv.tile([HD + 1, 512], F32, name=f"pvl{h2}", tag="pv")
        for h2 in range(2)
    ]
    for kt in range(NST):
        emit_pv_batch(last_ps, prev[0], kt, prev[2][kt])
    emit_evac(last_ps, prev[0], prev[1])


def build_program():
    nc = bacc.Bacc(
        "TRN2", target_bir_lowering=False, debug=False, num_devices=8
    )
    xT = nc.dram_tensor("xT", [D, S], BF16, kind="ExternalInput").ap()
    wqT = nc.dram_tensor("wqT", [D, DG], BF16, kind="ExternalInput").ap()
    wkT = nc.dram_tensor("wkT", [D, DG], BF16, kind="ExternalInput").ap()
    wvT = nc.dram_tensor("wvT", [D, DG], BF16, kind="ExternalInput").ap()
    bq = nc.dram_tensor("bq", [P, 4], F32, kind="ExternalInput").ap()
    bk = nc.dram_tensor("bk", [P, 4], F32, kind="ExternalInput").ap()
    bv = nc.dram_tensor("bv", [P, DG], BF16, kind="ExternalInput").ap()
    out = nc.dram_tensor("out", [LH, HD + 1, S], F32, kind="ExternalOutput").ap()

    with tile.TileContext(nc) as tc, ExitStack() as ctx:
        _build_attention(tc, ctx, (xT, wqT, wkT, wvT, bq, bk, bv, out))
    nc.compile()
    return nc


def make_in_maps(x, Wq, bq, Wk, bk, Wv, bv):
    bf = ml_dtypes.bfloat16
    x = np.asarray(x, np.float32)
    in_maps = []
    for c in range(8):
        b, g = c // 2, c % 2
        sl = slice(DG * g, DG * (g + 1))
        in_maps.append(
            {
                "xT": np.ascontiguousarray(x[b].T).astype(bf),
                "wqT": np.ascontiguousarray(np.asarray(Wq, np.float32)[sl].T).astype(bf),
                "wkT": np.ascontiguousarray(np.asarray(Wk, np.float32)[sl].T).astype(bf),
                "wvT": np.ascontiguousarray(np.asarray(Wv, np.float32)[sl].T).astype(bf),
                "bq": np.ascontiguousarray(np.asarray(bq, np.float32)[sl].reshape(4, P).T),
                "bk": np.ascontiguousarray(np.asarray(bk, np.float32)[sl].reshape(4, P).T),
                "bv": np.ascontiguousarray(
                    np.broadcast_to(np.asarray(bv, np.float32)[sl], (P, DG))
                ).astype(bf),
            }
        )
    return in_maps


def assemble(outs):
    res = np.empty((B, S, D), np.float32)
    for c in range(8):
        b, g = c // 2, c % 2
        o = np.asarray(outs[c], np.float32)       # [8, 65, 2048]
        hn = o[:, :HD, :] / o[:, HD : HD + 1, :]  # normalize softmax
        res[b, :, DG * g : DG * (g + 1)] = hn.transpose(2, 0, 1).reshape(S, DG)
    return res


_NC_CACHE = None


def _get_program():
    global _NC_CACHE
    if _NC_CACHE is None:
        _NC_CACHE = build_program()
    return _NC_CACHE


def _install_ntff_hook():
    """The agent image lacks ``antenv.axon_hooks``; recreate it and install
    the ctypes NTFF-profiling hook against libaxon_pjrt.so (the same thing
    trn_boot does when the module exists). Only used for trace=True runs."""
    import contextlib
    import ctypes
    import types

    try:
        from antenv.axon_hooks import get_axon_ntff_profile_hook  # noqa: F401

        return
    except ImportError:
        pass

    so_path = "/opt/axon/libaxon_pjrt.so"
    lib = ctypes.CDLL(so_path)
    if not hasattr(lib, "axon_start_nrt_profile"):
        return
    lib.axon_start_nrt_profile.argtypes = [
        ctypes.POINTER(ctypes.c_int64),
        ctypes.c_size_t,
    ]
    lib.axon_start_nrt_profile.restype = ctypes.c_int64
    lib.axon_stop_nrt_profile.argtypes = [ctypes.c_char_p]
    lib.axon_stop_nrt_profile.restype = ctypes.c_int64

    @contextlib.contextmanager
    def _hook(output_dir, device_ids):
        import jax

        jax.devices()
        if device_ids:
            ids = (ctypes.c_int64 * len(device_ids))(*device_ids)
            rc = lib.axon_start_nrt_profile(ids, len(device_ids))
        else:
            rc = lib.axon_start_nrt_profile(None, 0)
        if rc != 0:
            raise RuntimeError(f"axon_start_nrt_profile rc={rc}")
        try:
            yield
        finally:
            n = lib.axon_stop_nrt_profile(str(output_dir).encode())
            print(f"ntff profile: {n} file(s) written to {output_dir}")

    mod = types.ModuleType("antenv.axon_hooks")
    mod._hook = _hook
    mod.set_axon_ntff_profile_hook = lambda h: setattr(mod, "_hook", h)
    mod.get_axon_ntff_profile_hook = lambda: mod._hook
    sys.modules["antenv.axon_hooks"] = mod

    # artifact upload reaches for a shared bucket that this container can't
    # see; the local tmpdir is all the profile pipeline needs
    import concourse.bass_utils as bu

    bu.upload_artifacts = lambda tmpdir: tmpdir


def kernel(x, Wq, bq, Wk, bk, Wv, bv, trace=False, tmpdir=None):
    nc = _get_program()
    if trace:
        _install_ntff_hook()
    in_maps = make_in_maps(x, Wq, bq, Wk, bk, Wv, bv)
    res = run_bass_kernel_spmd(
        nc, in_maps, core_ids=list(range(8)), trace=trace, tmpdir=tmpdir
    )
    full = assemble([res.results[c]["out"] for c in range(8)])
    if trace:
        kernel.last_results = res
    return full
